# revision 14
# baseline (speedup 1.0000x reference)
"""Trainium2 Bass kernel for the delta-rule memory recurrence (DeltaNet-style).

Full-input contract: kernel(memory, key, value) -> final memory, all np.ndarray,
shapes (16,256,256), (16,4096,256), (16,4096,256) -> (16,256,256) float32.

Strategy: pure data-parallel over batch (2 batches per NeuronCore x 8 cores).
Per batch the sequential recurrence

    kn   = k_t / ||k_t||
    M   <- M - (1.1 * M kn - 0.1 * v_t) kn^T

is reformulated chunkwise (C=128 steps per chunk) via the WY / UT transform:

    A  = Kn Kn^T                      (C x C Gram of normalized keys)
    L  = 1.1 * strict_lower(A)
    Tinv = (I + L)^{-1}               (unit lower triangular inverse)
    H  = Tinv @ (-1.1 * Kn Mt + 0.1 * V)
    Mt <- Mt + Kn^T H                 (Mt = M^T state, (DK, DV))

(I+L)^{-1} is computed exactly with the nilpotent factorization
(I-L)(I+L^2)(I+L^4)(I+L^8)  [L^16 and beyond are numerically zero here].
Inversion machinery runs in fp16 matmuls (full PE rate, 10-bit mantissa),
state-path matmuls run as float32r (full rate at N>=256).
"""

import numpy as np

import concourse.bass as bass
import concourse.mybir as mybir
import concourse.tile as tile
from concourse.bass import ts
from concourse.bass_utils import run_bass_kernel_spmd
from concourse.masks import make_identity

F32 = mybir.dt.float32
F32R = mybir.dt.float32r
F16 = mybir.dt.float16
AOP = mybir.AluOpType
AFT = mybir.ActivationFunctionType

B, S, DK, DV = 16, 4096, 256, 256
NCORES = 8
BLOC = B // NCORES          # batches per core
C = 128                     # chunk length
LR = 0.1
AC = 1.0 + LR               # 1.1
NLEV = 3                    # squaring levels: (I-L)(I+L^2)(I+L^4)(I+L^8)


def _split_waits(nc, max_waits=1):
    """walrus codegen on this toolchain encodes at most one semaphore wait per
    instruction; hoist excess waits onto same-engine NoOps placed just before."""
    n_split = 0
    for f in nc.m.functions:
        for bb in f.blocks:
            insts = bb.instructions
            out = []
            for inst in insts:
                si = getattr(inst, "sync_info", None)
                w = list(si.on_wait) if (si and si.on_wait) else []
                k = 0
                while len(w) > max_waits:
                    head, w = w[:max_waits], w[max_waits:]
                    out.append(mybir.InstNoOp(
                        name=f"{inst.name}-wsplit{k}",
                        engine=inst.engine,
                        sync_info=mybir.SyncInfo(on_wait=head, on_update=[]),
                    ))
                    n_split += 1
                    k += 1
                if k:
                    inst.sync_info = mybir.SyncInfo(
                        on_wait=w, on_update=list(si.on_update or [])
                    )
                out.append(inst)
            bb.instructions = out
    return n_split


def build_nc(s_loc=S, state_mm_dtype=F32R, split=True):
    nch = s_loc // C
    nc = bass.Bass()
    memT = nc.declare_dram_parameter("memT", [BLOC, DK, DV], F32, isOutput=False)
    key_d = nc.declare_dram_parameter("key", [BLOC, s_loc, DK], F32, isOutput=False)
    val_d = nc.declare_dram_parameter("value", [BLOC, s_loc, DV], F32, isOutput=False)
    outT = nc.declare_dram_parameter("outT", [BLOC, DK, DV], F32, isOutput=True)

    SMM = state_mm_dtype  # state-path matmul tiles (float32r: full-rate fp32-ish mm)

    with tile.TileContext(nc) as tc:
        with (
            tc.tile_pool(name="consts", bufs=1) as consts,
            tc.tile_pool(name="kv", bufs=4) as kv,
            tc.tile_pool(name="norm", bufs=4) as normp,
            tc.tile_pool(name="kt", bufs=3) as ktp,
            tc.tile_pool(name="inv", bufs=3) as invp,
            tc.tile_pool(name="state", bufs=3) as statep,
            tc.tile_pool(name="mt", bufs=2) as mtp,
            tc.tile_pool(name="ps_inv", bufs=4, space="PSUM") as ps_inv,
            tc.tile_pool(name="ps_state", bufs=2, space="PSUM") as ps_state,
            tc.tile_pool(name="ps_upd", bufs=2, space="PSUM") as ps_upd,
        ):
            ident32 = consts.tile([128, 128], F32, tag="ident32")
            make_identity(nc, ident32)
            ident16 = consts.tile([128, 128], F16, tag="ident16")
            make_identity(nc, ident16)
            # paired identity (both halves) for G0 = I + LTn
            i2_16 = consts.tile([128, 2, 128], F16, tag="i2_16")
            nc.gpsimd.memset(i2_16, 0.0)
            nc.gpsimd.affine_select(
                out=i2_16, in_=i2_16, compare_op=AOP.not_equal, fill=1.0,
                base=0, pattern=[[0, 2], [-1, 128]], channel_multiplier=1,
            )

            # initial state Mt (= M^T) per batch, laid out [p, ktile, v]
            mt = []
            for b in range(BLOC):
                t0 = mtp.tile([128, 2, DV], F32, tag=f"mt0f{b}")
                nc.sync.dma_start(
                    out=t0, in_=memT[b].rearrange("(j p) v -> p j v", p=128)
                )
                t = mtp.tile([128, 2, DV], SMM, tag=f"mt{b}")
                nc.vector.tensor_copy(t, t0)
                mt.append(t)

            for c in range(nch):
                Kt, Vt, Kn = [], [], []
                for b in range(BLOC):
                    k = kv.tile([128, DK], F32, tag=f"k{b}")
                    nc.sync.dma_start(out=k, in_=key_d[b, c * C:(c + 1) * C, :])
                    v = kv.tile([128, DV], F32, tag=f"v{b}")
                    nc.sync.dma_start(out=v, in_=val_d[b, c * C:(c + 1) * C, :])
                    Kt.append(k)
                    Vt.append(v)
                # normalization: ssq/rn for both batches share [128, 2] tiles
                ssq = normp.tile([128, 2], F32, tag="ssq")
                for b in range(BLOC):
                    scr = normp.tile([128, DK], F32, tag="scr")
                    nc.scalar.activation(out=scr, in_=Kt[b], func=AFT.Square,
                                         accum_out=ssq[:, b:b + 1])
                nrm = normp.tile([128, 2], F32, tag="nrm")
                nc.scalar.activation(nrm, ssq, AFT.Sqrt)
                rn = normp.tile([128, 2], F32, tag="rn")
                nc.vector.reciprocal(rn, nrm)
                for b in range(BLOC):
                    kn = normp.tile([128, DK], SMM, tag=f"kn{b}")
                    nc.vector.tensor_scalar_mul(kn, Kt[b], rn[:, b:b + 1])
                    Kn.append(kn)

                # transposes of Kn: KnTs (f32r, state path) + KnTh (f16, for A)
                # batch-paired psum: [128, b, 128] per k-tile j
                KnTs = [None] * 2
                KnTh = [None] * 2
                for j in range(2):
                    tp = ps_state.tile([128, 2, 128], F32, tag="st")
                    for b in range(BLOC):
                        nc.tensor.transpose(
                            tp[:, b, :], Kn[b][:, ts(j, 128)].bitcast(F32),
                            ident32)
                    s32 = ktp.tile([128, 2, 128], SMM, tag=f"knts{j}")
                    nc.scalar.copy(s32, tp)
                    s16 = ktp.tile([128, 2, 128], F16, tag=f"knth{j}")
                    nc.vector.tensor_copy(s16, tp)
                    KnTs[j] = s32
                    KnTh[j] = s16

                # A = Kn Kn^T (both batches share a [128, 2, 128] psum tile)
                a_ps = ps_inv.tile([128, 2, 128], F32, tag="inv")
                for b in range(BLOC):
                    for j in range(2):
                        nc.tensor.matmul(
                            a_ps[:, b, :], KnTh[j][:, b, :], KnTh[j][:, b, :],
                            start=(j == 0), stop=(j == 1),
                        )
                a_neg = invp.tile([128, 2, 128], F16, tag="a_neg")
                nc.scalar.mul(a_neg, a_ps, -AC)
                # Ln = -L = strict_lower(a_neg); LTn = -L^T = strict_upper(a_neg)
                ln = invp.tile([128, 2, 128], F16, tag="ln")
                nc.gpsimd.affine_select(
                    out=ln, in_=a_neg, compare_op=AOP.is_gt, fill=0.0,
                    base=0, pattern=[[0, 2], [-1, 128]], channel_multiplier=1,
                )
                ltn = invp.tile([128, 2, 128], F16, tag="ltn")
                nc.gpsimd.affine_select(
                    out=ltn, in_=a_neg, compare_op=AOP.is_gt, fill=0.0,
                    base=0, pattern=[[0, 2], [1, 128]], channel_multiplier=-1,
                )

                # power chain; L/LT pairs packed into one [128, 2, 256] psum
                def pow_pair(lhsT_l, rhs_l, lhsT_lt, rhs_lt, tag, eng):
                    ps = ps_inv.tile([128, 2, 256], F32, tag="inv")
                    for b in range(BLOC):
                        nc.tensor.matmul(ps[:, b, 0:128],
                                         lhsT_l[:, b, :], rhs_l[:, b, :])
                        nc.tensor.matmul(ps[:, b, 128:256],
                                         lhsT_lt[:, b, :], rhs_lt[:, b, :])
                    sb = invp.tile([128, 2, 256], F16, tag=tag)
                    if eng == "v":
                        nc.vector.tensor_copy(sb, ps)
                    else:
                        nc.scalar.copy(sb, ps)
                    return sb[:, :, 0:128], sb[:, :, 128:256]

                # L2 = LTn^T@Ln, LT2 = Ln^T@LTn ; L4 = LT2^T... ; L8
                l2, lt2 = pow_pair(ltn, ln, ln, ltn, "p2", "v")
                l4, lt4 = pow_pair(lt2, l2, l2, lt2, "p4", "s")
                p8 = ps_inv.tile([128, 2, 128], F32, tag="inv")
                for b in range(BLOC):
                    nc.tensor.matmul(p8[:, b, :], lt4[:, b, :], l4[:, b, :])
                l8 = invp.tile([128, 2, 128], F16, tag="p8")
                nc.vector.tensor_copy(l8, p8)

                # G chain: G0 = I + LTn; G <- (I + LT^{2^i}) G via psum inject
                g = invp.tile([128, 2, 128], F16, tag="g0")
                nc.vector.tensor_add(g, i2_16, ltn)
                for i, lp in enumerate((l2, l4, l8)):
                    gp = ps_inv.tile([128, 2, 128], F32, tag="inv")
                    for b in range(BLOC):
                        nc.tensor.matmul(gp[:, b, :], lp[:, b, :], g[:, b, :],
                                         start=True, stop=False)
                        nc.tensor.matmul(gp[:, b, :], ident16, g[:, b, :],
                                         start=False, stop=True)
                    gn = invp.tile([128, 2, 128], F16, tag=f"g{i + 1}")
                    if i % 2 == 0:
                        nc.scalar.copy(gn, gp)
                    else:
                        nc.vector.tensor_copy(gn, gp)
                    g = gn

                # state path per batch
                for b in range(BLOC):
                    y_ps = ps_state.tile([128, DV], F32, tag="st")
                    for j in range(2):
                        nc.tensor.matmul(
                            y_ps, KnTs[j][:, b, :], mt[b][:, j, :],
                            start=(j == 0), stop=(j == 1),
                        )
                    # R' = 10*R = -11 Kn Mt + V  (fp16); the 0.1 folds into H
                    rh = statep.tile([128, DV], F16, tag=f"rh{b}")
                    nc.vector.scalar_tensor_tensor(
                        out=rh, in0=y_ps, scalar=-10.0 * AC, in1=Vt[b],
                        op0=AOP.mult, op1=AOP.add,
                    )
                    h_ps = ps_state.tile([128, DV], F32, tag="st")
                    nc.tensor.matmul(h_ps, g[:, b, :], rh)
                    h_sb = statep.tile([128, DV], SMM, tag=f"hs{b}")
                    nc.scalar.mul(h_sb, h_ps, LR)      # H = 0.1 * Tinv R'
                    u_ps = ps_upd.tile([128, 2, DV], F32, tag="u")
                    for j in range(2):
                        nc.tensor.matmul(
                            u_ps[:, j, :], Kn[b][:, ts(j, 128)], h_sb,
                        )
                    mt_new = mtp.tile([128, 2, DV], SMM, tag=f"mt{b}")
                    nc.vector.tensor_add(mt_new, mt[b], u_ps)
                    mt[b] = mt_new

            for b in range(BLOC):
                nc.sync.dma_start(
                    out=outT[b].rearrange("(j p) v -> p j v", p=128),
                    in_=mt[b].bitcast(F32),
                )
    if split:
        _split_waits(nc)
    return nc


_NC_CACHE = {}

# test-harness hooks (the grading harness just calls kernel())
TRACE = False
LAST_RESULT = None


def _get_nc(s_loc=S):
    if s_loc not in _NC_CACHE:
        _NC_CACHE[s_loc] = build_nc(s_loc)
    return _NC_CACHE[s_loc]


def kernel(memory, key, value):
    global LAST_RESULT
    memory = np.ascontiguousarray(np.asarray(memory), dtype=np.float32)
    key = np.ascontiguousarray(np.asarray(key), dtype=np.float32)
    value = np.ascontiguousarray(np.asarray(value), dtype=np.float32)
    s_loc = key.shape[1]
    nc = _get_nc(s_loc)
    memT = np.ascontiguousarray(memory.transpose(0, 2, 1))
    in_maps = []
    for i in range(NCORES):
        sl = slice(i * BLOC, (i + 1) * BLOC)
        in_maps.append({
            "memT": memT[sl],
            "key": np.ascontiguousarray(key[sl]),
            "value": np.ascontiguousarray(value[sl]),
        })
    res = run_bass_kernel_spmd(nc, in_maps, list(range(NCORES)), trace=TRACE)
    LAST_RESULT = res
    outs = [res.results[i]["outT"] for i in range(NCORES)]
    out = np.concatenate(outs, axis=0)          # (16, DK, DV) = M^T
    return np.ascontiguousarray(out.transpose(0, 2, 1))


# revision 15
# speedup vs baseline: 1.3073x; 1.3073x over previous
"""Trainium2 Bass kernel for the delta-rule memory recurrence (DeltaNet-style).

Full-input contract: kernel(memory, key, value) -> final memory, all np.ndarray,
shapes (16,256,256), (16,4096,256), (16,4096,256) -> (16,256,256) float32.

Strategy: pure data-parallel over batch (2 batches per NeuronCore x 8 cores).
Per batch the sequential recurrence

    kn   = k_t / ||k_t||
    M   <- M - (1.1 * M kn - 0.1 * v_t) kn^T

is reformulated chunkwise (C=128 steps per chunk) via the WY / UT transform:

    A  = Kn Kn^T                      (C x C Gram of normalized keys)
    L  = 1.1 * strict_lower(A)
    Tinv = (I + L)^{-1}               (unit lower triangular inverse)
    H  = Tinv @ (-1.1 * Kn Mt + 0.1 * V)
    Mt <- Mt + Kn^T H                 (Mt = M^T state, (DK, DV))

(I+L)^{-1} is computed exactly with the nilpotent factorization
(I-L)(I+L^2)(I+L^4)(I+L^8)  [L^16 and beyond are numerically zero here].
Inversion machinery runs in fp16 matmuls (full PE rate, 10-bit mantissa),
state-path matmuls run as float32r (full rate at N>=256).
"""

import numpy as np

import concourse.bass as bass
import concourse.mybir as mybir
import concourse.tile as tile
from concourse.bass import ts
from concourse.bass_utils import run_bass_kernel_spmd
from concourse.masks import make_identity

F32 = mybir.dt.float32
F32R = mybir.dt.float32r
F16 = mybir.dt.float16
AOP = mybir.AluOpType
AFT = mybir.ActivationFunctionType

B, S, DK, DV = 16, 4096, 256, 256
NCORES = 8
BLOC = B // NCORES          # batches per core
C = 128                     # chunk length
LR = 0.1
AC = 1.0 + LR               # 1.1
NLEV = 3                    # squaring levels: (I-L)(I+L^2)(I+L^4)(I+L^8)


def _split_waits(nc, max_waits=1):
    """walrus codegen on this toolchain encodes at most one semaphore wait per
    instruction; hoist excess waits onto same-engine NoOps placed just before."""
    n_split = 0
    for f in nc.m.functions:
        for bb in f.blocks:
            insts = bb.instructions
            out = []
            for inst in insts:
                si = getattr(inst, "sync_info", None)
                w = list(si.on_wait) if (si and si.on_wait) else []
                k = 0
                while len(w) > max_waits:
                    head, w = w[:max_waits], w[max_waits:]
                    out.append(mybir.InstNoOp(
                        name=f"{inst.name}-wsplit{k}",
                        engine=inst.engine,
                        sync_info=mybir.SyncInfo(on_wait=head, on_update=[]),
                    ))
                    n_split += 1
                    k += 1
                if k:
                    inst.sync_info = mybir.SyncInfo(
                        on_wait=w, on_update=list(si.on_update or [])
                    )
                out.append(inst)
            bb.instructions = out
    return n_split


def build_nc(s_loc=S, state_mm_dtype=F32R, split=True):
    nch = s_loc // C
    nc = bass.Bass()
    memT = nc.declare_dram_parameter("memT", [BLOC, DK, DV], F32, isOutput=False)
    key_d = nc.declare_dram_parameter("key", [BLOC, s_loc, DK], F32, isOutput=False)
    val_d = nc.declare_dram_parameter("value", [BLOC, s_loc, DV], F32, isOutput=False)
    outT = nc.declare_dram_parameter("outT", [BLOC, DK, DV], F32, isOutput=True)

    SMM = state_mm_dtype  # state-path matmul tiles (float32r: full-rate fp32-ish mm)

    with tile.TileContext(nc) as tc:
        with (
            tc.tile_pool(name="consts", bufs=1) as consts,
            tc.tile_pool(name="kv", bufs=4) as kv,
            tc.tile_pool(name="norm", bufs=4) as normp,
            tc.tile_pool(name="kt", bufs=3) as ktp,
            tc.tile_pool(name="inv", bufs=3) as invp,
            tc.tile_pool(name="state", bufs=3) as statep,
            tc.tile_pool(name="mt", bufs=3) as mtp,
            tc.tile_pool(name="ps_inv", bufs=4, space="PSUM") as ps_inv,
            tc.tile_pool(name="ps_state", bufs=2, space="PSUM") as ps_state,
            tc.tile_pool(name="ps_upd", bufs=2, space="PSUM") as ps_upd,
        ):
            ident32 = consts.tile([128, 128], F32, tag="ident32")
            make_identity(nc, ident32)
            ident16 = consts.tile([128, 128], F16, tag="ident16")
            make_identity(nc, ident16)
            # paired identity (both halves) for G0 = I + LTn
            i2_16 = consts.tile([128, 2, 128], F16, tag="i2_16")
            nc.gpsimd.memset(i2_16, 0.0)
            nc.gpsimd.affine_select(
                out=i2_16, in_=i2_16, compare_op=AOP.not_equal, fill=1.0,
                base=0, pattern=[[0, 2], [-1, 128]], channel_multiplier=1,
            )

            # initial state Mt (= M^T) per batch, laid out [p, ktile, v]
            mt = []
            for b in range(BLOC):
                t0 = mtp.tile([128, 2, DV], F32, tag=f"mt0f{b}")
                nc.sync.dma_start(
                    out=t0, in_=memT[b].rearrange("(j p) v -> p j v", p=128)
                )
                t = mtp.tile([128, 2, DV], SMM, tag=f"mt{b}")
                nc.vector.tensor_copy(t, t0)
                mt.append(t)

            def emit_precomp(c):
                Kt, Vt, Kn = [], [], []
                for b in range(BLOC):
                    k = kv.tile([128, DK], F32, tag=f"k{b}")
                    nc.sync.dma_start(out=k, in_=key_d[b, c * C:(c + 1) * C, :])
                    v = kv.tile([128, DV], F32, tag=f"v{b}")
                    nc.sync.dma_start(out=v, in_=val_d[b, c * C:(c + 1) * C, :])
                    Kt.append(k)
                    Vt.append(v)
                # normalization: ssq/rn for both batches share [128, 2] tiles
                ssq = normp.tile([128, 2], F32, tag="ssq")
                for b in range(BLOC):
                    scr = normp.tile([128, DK], F32, tag="scr")
                    nc.scalar.activation(out=scr, in_=Kt[b], func=AFT.Square,
                                         accum_out=ssq[:, b:b + 1])
                nrm = normp.tile([128, 2], F32, tag="nrm")
                nc.scalar.activation(nrm, ssq, AFT.Sqrt)
                rn = normp.tile([128, 2], F32, tag="rn")
                nc.vector.reciprocal(rn, nrm)
                for b in range(BLOC):
                    kn = normp.tile([128, DK], SMM, tag=f"kn{b}")
                    nc.vector.tensor_scalar_mul(kn, Kt[b], rn[:, b:b + 1])
                    Kn.append(kn)

                # transposes of Kn: KnTs (f32r, state path) + KnTh (f16, for A)
                # batch-paired psum: [128, b, 128] per k-tile j
                KnTs = [None] * 2
                KnTh = [None] * 2
                for j in range(2):
                    tp = ps_inv.tile([128, 2, 128], F32, tag="inv")
                    for b in range(BLOC):
                        nc.tensor.transpose(
                            tp[:, b, :], Kn[b][:, ts(j, 128)].bitcast(F32),
                            ident32)
                    s32 = ktp.tile([128, 2, 128], SMM, tag=f"knts{j}")
                    nc.scalar.copy(s32, tp)
                    s16 = ktp.tile([128, 2, 128], F16, tag=f"knth{j}")
                    nc.vector.tensor_copy(s16, tp)
                    KnTs[j] = s32
                    KnTh[j] = s16

                # A = Kn Kn^T (both batches share a [128, 2, 128] psum tile)
                a_ps = ps_inv.tile([128, 2, 128], F32, tag="inv")
                for b in range(BLOC):
                    for j in range(2):
                        nc.tensor.matmul(
                            a_ps[:, b, :], KnTh[j][:, b, :], KnTh[j][:, b, :],
                            start=(j == 0), stop=(j == 1),
                        )
                a_neg = invp.tile([128, 2, 128], F16, tag="a_neg")
                nc.scalar.mul(a_neg, a_ps, -AC)
                # Ln = -L = strict_lower(a_neg); LTn = -L^T = strict_upper(a_neg)
                ln = invp.tile([128, 2, 128], F16, tag="ln")
                nc.gpsimd.affine_select(
                    out=ln, in_=a_neg, compare_op=AOP.is_gt, fill=0.0,
                    base=0, pattern=[[0, 2], [-1, 128]], channel_multiplier=1,
                )
                ltn = invp.tile([128, 2, 128], F16, tag="ltn")
                nc.gpsimd.affine_select(
                    out=ltn, in_=a_neg, compare_op=AOP.is_gt, fill=0.0,
                    base=0, pattern=[[0, 2], [1, 128]], channel_multiplier=-1,
                )

                # power chain; L/LT pairs packed into one [128, 2, 256] psum
                def pow_pair(lhsT_l, rhs_l, lhsT_lt, rhs_lt, tag, eng):
                    ps = ps_inv.tile([128, 2, 256], F32, tag="inv")
                    for b in range(BLOC):
                        nc.tensor.matmul(ps[:, b, 0:128],
                                         lhsT_l[:, b, :], rhs_l[:, b, :])
                        nc.tensor.matmul(ps[:, b, 128:256],
                                         lhsT_lt[:, b, :], rhs_lt[:, b, :])
                    sb = invp.tile([128, 2, 256], F16, tag=tag)
                    if eng == "v":
                        nc.vector.tensor_copy(sb, ps)
                    else:
                        nc.scalar.copy(sb, ps)
                    return sb[:, :, 0:128], sb[:, :, 128:256]

                # L2 = LTn^T@Ln, LT2 = Ln^T@LTn ; L4 = LT2^T... ; L8
                l2, lt2 = pow_pair(ltn, ln, ln, ltn, "p2", "v")
                l4, lt4 = pow_pair(lt2, l2, l2, lt2, "p4", "s")
                p8 = ps_inv.tile([128, 2, 128], F32, tag="inv")
                for b in range(BLOC):
                    nc.tensor.matmul(p8[:, b, :], lt4[:, b, :], l4[:, b, :])
                l8 = invp.tile([128, 2, 128], F16, tag="p8")
                nc.vector.tensor_copy(l8, p8)

                # G chain: G0 = I + LTn; G <- (I + LT^{2^i}) G via psum inject
                g = invp.tile([128, 2, 128], F16, tag="g0")
                nc.vector.tensor_add(g, i2_16, ltn)
                for i, lp in enumerate((l2, l4, l8)):
                    gp = ps_inv.tile([128, 2, 128], F32, tag="inv")
                    gn = invp.tile([128, 2, 128], F16, tag=f"g{i + 1}")
                    if i == 1:
                        # G' = G + LT^4 G : matmul + DVE add (no inject)
                        for b in range(BLOC):
                            nc.tensor.matmul(gp[:, b, :], lp[:, b, :], g[:, b, :])
                        nc.vector.tensor_add(gn, g, gp)
                    else:
                        for b in range(BLOC):
                            nc.tensor.matmul(gp[:, b, :], lp[:, b, :], g[:, b, :],
                                             start=True, stop=False)
                            nc.tensor.matmul(gp[:, b, :], ident16, g[:, b, :],
                                             start=False, stop=True)
                        nc.scalar.copy(gn, gp)
                    g = gn

                return dict(Kn=Kn, Vt=Vt, KnTs=KnTs, g=g)

            def emit_state(art):
                Kn, Vt, KnTs, g = art["Kn"], art["Vt"], art["KnTs"], art["g"]
                for b in range(BLOC):
                    y_ps = ps_state.tile([128, DV], F32, tag="st")
                    for j in range(2):
                        nc.tensor.matmul(
                            y_ps, KnTs[j][:, b, :], mt[b][:, j, :],
                            start=(j == 0), stop=(j == 1),
                        )
                    # R' = 10*R = -11 Kn Mt + V  (fp16); the 0.1 folds into H
                    rh = statep.tile([128, DV], F16, tag=f"rh{b}")
                    nc.vector.scalar_tensor_tensor(
                        out=rh, in0=y_ps, scalar=-10.0 * AC, in1=Vt[b],
                        op0=AOP.mult, op1=AOP.add,
                    )
                    h_ps = ps_state.tile([128, DV], F32, tag="st")
                    nc.tensor.matmul(h_ps, g[:, b, :], rh)
                    h_sb = statep.tile([128, DV], SMM, tag=f"hs{b}")
                    nc.scalar.mul(h_sb, h_ps, LR)      # H = 0.1 * Tinv R'
                    u_ps = ps_upd.tile([128, 2, DV], F32, tag="u")
                    for j in range(2):
                        nc.tensor.matmul(
                            u_ps[:, j, :], Kn[b][:, ts(j, 128)], h_sb,
                        )
                    mt_new = mtp.tile([128, 2, DV], SMM, tag=f"mt{b}")
                    nc.vector.tensor_add(mt_new, mt[b], u_ps)
                    mt[b] = mt_new

            # software pipeline: chunk c+1's state-independent precompute is
            # emitted before chunk c's state path so the PE always has
            # independent work while psum->sbuf copies drain.
            art = emit_precomp(0)
            for c in range(nch):
                nxt = emit_precomp(c + 1) if c + 1 < nch else None
                emit_state(art)
                art = nxt

            for b in range(BLOC):
                nc.sync.dma_start(
                    out=outT[b].rearrange("(j p) v -> p j v", p=128),
                    in_=mt[b].bitcast(F32),
                )
    if split:
        _split_waits(nc)
    return nc


_NC_CACHE = {}

# test-harness hooks (the grading harness just calls kernel())
TRACE = False
LAST_RESULT = None


def _get_nc(s_loc=S):
    if s_loc not in _NC_CACHE:
        _NC_CACHE[s_loc] = build_nc(s_loc)
    return _NC_CACHE[s_loc]


def kernel(memory, key, value):
    global LAST_RESULT
    memory = np.ascontiguousarray(np.asarray(memory), dtype=np.float32)
    key = np.ascontiguousarray(np.asarray(key), dtype=np.float32)
    value = np.ascontiguousarray(np.asarray(value), dtype=np.float32)
    s_loc = key.shape[1]
    nc = _get_nc(s_loc)
    memT = np.ascontiguousarray(memory.transpose(0, 2, 1))
    in_maps = []
    for i in range(NCORES):
        sl = slice(i * BLOC, (i + 1) * BLOC)
        in_maps.append({
            "memT": memT[sl],
            "key": np.ascontiguousarray(key[sl]),
            "value": np.ascontiguousarray(value[sl]),
        })
    res = run_bass_kernel_spmd(nc, in_maps, list(range(NCORES)), trace=TRACE)
    LAST_RESULT = res
    outs = [res.results[i]["outT"] for i in range(NCORES)]
    out = np.concatenate(outs, axis=0)          # (16, DK, DV) = M^T
    return np.ascontiguousarray(out.transpose(0, 2, 1))
